# revision 53
# baseline (speedup 1.0000x reference)
"""Trainium2 Bass kernel for memory-augmented causal attention.

Reference computation (b=2, n=1024, m=1024 memory, 16 heads, d_head=64):
  q = (x @ Wq) * scale ; k,v = split(x @ Wkv) ; k = [mem_k; k] ; v = [mem_v; v]
  sim = q k^T + pos_bias ; causal mask on self part ; softmax ; out = attn v
  return out @ Wo + bo

Sharding: 16 heads across 8 cores (2 heads/core), both batches on every core
(pos_bias reused across batches on-chip).  Each core computes a partial
output (its heads' contribution through Wo rows); host sums the 8 partials.

All matmuls run as float32r (full-rate fp32 on TRN2 PE, ~1e-4 rms rounding).
pos_bias is pre-transposed/masked/bf16 on host and added to the logits in
PSUM via a bf16 identity matmul.  exp on ScalarE.  Softmax denominators come
from a ones-column appended to V (row 64 of the AV accumulation).
"""

import numpy as np
import ml_dtypes

import concourse.bass as bass
import concourse.mybir as mybir
import concourse.tile as tile
from concourse import bacc
from concourse import bass_utils
from concourse.masks import make_identity

F32 = mybir.dt.float32
F32R = mybir.dt.float32r
BF16 = mybir.dt.bfloat16
F16 = mybir.dt.float16

HEADS = 16
DH = 64               # head dim
B = 2                 # batch
N = 1024              # query length
M = 1024              # memory length
JT = N + M            # total key length
DIM = 1024
SCALE = DH ** -0.5
NCORE = 8
HPC = HEADS // NCORE  # heads per core = 2
NEG = -1.0e9          # mask value (exp -> 0 in fp32)

NKC = DIM // 128      # contraction chunks for projections = 8
NJ = JT // 128        # j chunks = 16
NJ_MEM = M // 128     # memory j chunks = 8
NIC = N // 512        # i chunks of 512 = 2


def _self_chunks(ic):
    # self j-chunk k (j0 = 1024 + 128k) unmasked for i-chunk ic iff
    # j0 <= 1023 + ic*512 + 1024  ->  128k <= ic*512 + 511
    return (ic * 512 + 511) // 128 + 1


def _unmasked_jcs(ic):
    return list(range(NJ_MEM)) + [NJ_MEM + k for k in range(min(8, _self_chunks(ic)))]


_NC_CACHE = None


def _build():
    global _NC_CACHE
    if _NC_CACHE is not None:
        return _NC_CACHE

    nc = bacc.Bacc("TRN2", target_bir_lowering=False, debug=False)

    XT = nc.dram_tensor("xT", [B, DIM, N], F16, kind="ExternalInput").ap()
    WQ = nc.dram_tensor("wq", [DIM, 128], F16, kind="ExternalInput").ap()
    WK = nc.dram_tensor("wk", [DIM, 128], F16, kind="ExternalInput").ap()
    WV = nc.dram_tensor("wv", [DIM, 128], F16, kind="ExternalInput").ap()
    WO = nc.dram_tensor("wo", [128, DIM], F32R, kind="ExternalInput").ap()
    MKT = nc.dram_tensor("mkT", [B, 128, M], F16, kind="ExternalInput").ap()
    MV = nc.dram_tensor("mv", [B, HPC, NJ_MEM, 128, DH + 1], F16,
                        kind="ExternalInput").ap()
    BIAST = nc.dram_tensor("biasT", [HPC, JT, N], BF16, kind="ExternalInput").ap()
    ONES = nc.dram_tensor("ones_self", [128, HPC, NJ - NJ_MEM, 1], F16,
                          kind="ExternalInput").ap()
    OUT = nc.dram_tensor("out", [B, N, DIM], F16, kind="ExternalOutput").ap()

    with tile.TileContext(nc) as tc:
        with tc.tile_pool(name="const", bufs=1) as cp, \
             tc.tile_pool(name="wts", bufs=1) as wp, \
             tc.tile_pool(name="xtp", bufs=16) as xtp, \
             tc.tile_pool(name="big", bufs=1) as bigp, \
             tc.tile_pool(name="stage", bufs=1) as stp, \
             tc.tile_pool(name="biasp", bufs=40) as biasp, \
             tc.tile_pool(name="expp", bufs=16) as expp, \
             tc.tile_pool(name="outst", bufs=4) as outstp, \
             tc.tile_pool(name="smalls", bufs=3) as smallp, \
             tc.tile_pool(name="psum", bufs=1, space="PSUM") as psp:

            # ---- constants ----
            identb = cp.tile([128, 128], BF16)
            make_identity(nc, identb[:])
            identf = cp.tile([128, 128], F32)
            make_identity(nc, identf[:])
            identr = cp.tile([128, 128], F32R)
            nc.vector.tensor_copy(identr[:], identf[:])
            identh = cp.tile([128, 128], F16)
            nc.vector.tensor_copy(identh[:], identf[:])

            # ---- weights (scalar queue; sync busy with xT) ----
            wq_t = wp.tile([128, NKC * 128], F16, tag="wqo")
            wk_t = wp.tile([128, NKC * 128], F16)
            wv_t = wp.tile([128, NKC * 128], F16)
            wo_t = wp.tile([128, DIM], F32R, tag="wqo")
            def load_w(tl, src):
                nc.scalar.dma_start(
                    tl[:].rearrange("p (kc m) -> p kc m", m=128),
                    src.rearrange("(kc p) m -> p kc m", p=128))
            load_w(wq_t, WQ)

            # ---- persistent per-batch tensors ----
            qT = [bigp.tile([128, N], F16, name=f"qT{b}") for b in range(B)]
            kT = [bigp.tile([128, JT], F16, name=f"kT{b}") for b in range(B)]
            vaug = [bigp.tile([128, HPC * NJ * (DH + 1)], F16, name=f"vaug{b}")
                    for b in range(B)]

            def vaug_slice(b, h, jc):
                o = (h * NJ + jc) * (DH + 1)
                return vaug[b][:, o:o + DH + 1]
            outT = [bigp.tile([128, N], F32R, name=f"outT{b}") for b in range(B)]

            # =============== Phase 1: projections ===============
            copy_idx = 0

            def copy_balanced(out_ap, in_ap, eng=None):
                nonlocal copy_idx
                if eng is None:
                    eng = "v" if copy_idx % 2 == 0 else "s"
                    copy_idx += 1
                if eng == "v":
                    nc.vector.tensor_copy(out_ap, in_ap)
                else:
                    nc.scalar.copy(out_ap, in_ap)

            def warm(n, tag="smallps", width=128):
                # dummy matmuls on resident constants; positioned before a
                # known PE stall they keep the HAM clock at 2.4 GHz
                wps = psp.tile([128, width], F32, name="warmps", tag=tag,
                               bufs=4 if tag == "smallps" else 2)
                for _ in range(n):
                    nc.tensor.matmul(wps[:, 0:128], identr[:], identr[:],
                                     start=True, stop=True,
                                     skip_group_check=True)

            # preload all xT tiles for both batches (sync + scalar queues);
            # each weight is queued on scalar right before the batch needing it
            xts = {}
            for b in range(B):
                for kc in range(NKC):
                    t = xtp.tile([128, N], F16, name=f"xt{b}_{kc}", tag="xt")
                    eng = nc.sync if (kc % 2 == 0) else nc.scalar
                    eng.dma_start(t[:], XT[b, kc * 128:(kc + 1) * 128, :])
                    xts[(b, kc)] = t
                if b == 0:
                    load_w(wk_t, WK)
            load_w(wv_t, WV)
            nc.scalar.dma_start(wo_t[:], WO)

            for b in range(B):
                # mem parts straight from DRAM
                nc.sync.dma_start(kT[b][:, 0:M], MKT[b])
                for h in range(HPC):
                    nc.gpsimd.dma_start(
                        vaug[b][:].rearrange(
                            "p (h jc x) -> p h jc x", h=HPC, x=DH + 1)[:, h, 0:NJ_MEM],
                        MV[b, h].rearrange("jc p x -> p jc x"))
                # ones columns for the self chunks, via strided DMA
                for h in range(HPC):
                    nc.gpsimd.dma_start(
                        vaug[b][:].rearrange(
                            "p (s x) -> p s x", x=DH + 1)[
                            :, h * NJ + NJ_MEM:h * NJ + NJ, DH:DH + 1],
                        ONES[:, h])

            def proj_qk(kind, b):
                wt = wq_t if kind == "q" else wk_t
                ps = psp.tile([128, N], F32, name="projps", tag="simps", bufs=2)
                for icx in range(NIC):
                    for kc in range(NKC):
                        nc.tensor.matmul(
                            ps[:, icx * 512:(icx + 1) * 512],
                            wt[:, kc * 128:(kc + 1) * 128],
                            xts[(b, kc)][:, icx * 512:(icx + 1) * 512],
                            start=(kc == 0), stop=(kc == NKC - 1))
                if kind == "q":
                    copy_balanced(qT[b][:], ps[:], eng="v")
                else:
                    copy_balanced(kT[b][:, M:JT], ps[:], eng="v")

            def proj_v(b):
                # accumulate in [128,512] halves on smallps so the sims'
                # psum slots stay free (runs interleaved with attention)
                vst = stp.tile([128, N], F16, name="vstage")
                for icx in range(NIC):
                    ps = psp.tile([128, 512], F32, name="vps", tag="smallps",
                                  bufs=4)
                    for kc in range(NKC):
                        nc.tensor.matmul(
                            ps[:],
                            wv_t[:, kc * 128:(kc + 1) * 128],
                            xts[(b, kc)][:, icx * 512:(icx + 1) * 512],
                            start=(kc == 0), stop=(kc == NKC - 1))
                    copy_balanced(vst[:, icx * 512:(icx + 1) * 512], ps[:],
                                  eng="v")
                for jb in range(8):
                    tp = psp.tile([128, 128], F16, name="tps",
                                  tag="smallps", bufs=4)
                    nc.tensor.transpose(
                        tp[:], vst[:, jb * 128:(jb + 1) * 128], identh[:])
                    jc = NJ_MEM + jb
                    dst = vaug[b][:].rearrange(
                        "p (h jjc x) -> p h jjc x", h=HPC, x=DH + 1)[
                        :, :, jc, 0:DH]
                    copy_balanced(
                        dst, tp[:].rearrange("p (h x) -> p h x", h=HPC),
                        eng="v")

            warm(32)
            for kind, b in (("q", 0), ("k", 0), ("q", 1), ("k", 1)):
                proj_qk(kind, b)
                warm(4)
            proj_v(0)
            proj_v(1)

            # =============== Phase 2 + 3 interleaved ===============
            def out_proj_half(b, ib):
                ob = outstp.tile([128, DIM], F16, name="ob")
                for dc in range(DIM // 512):
                    ps = psp.tile([128, 512], F32, name="ops", tag="smallps",
                                  bufs=4)
                    nc.tensor.matmul(
                        ps[:],
                        outT[b][:, ib * 128:(ib + 1) * 128],
                        wo_t[:, dc * 512:(dc + 1) * 512],
                        start=True, stop=True)
                    copy_balanced(ob[:, dc * 512:(dc + 1) * 512], ps[:],
                                  eng="v")
                nc.sync.dma_start(OUT[b, ib * 128:(ib + 1) * 128, :], ob[:])

            for ic in range(NIC):
                jcs = _unmasked_jcs(ic)
                av = []  # allocated lazily at the first AV emission
                pend = [[] for _ in range(HPC)]  # AV two iterations behind

                def do_av(h, p, last):
                    if not av:
                        for hh in range(HPC):
                            av.append([psp.tile(
                                [DH + 1, 512], F32, name=f"av{hh}_{bb}",
                                tag="smallps", bufs=4) for bb in range(B)])
                    expt_, jc_, idx = p
                    for b in range(B):
                        nc.tensor.matmul(
                            av[h][b][:],
                            vaug_slice(b, h, jc_),
                            expt_[:, b * 512:(b + 1) * 512],
                            start=(idx == 0), stop=last,
                            skip_group_check=True)

                for jj, jc in enumerate(jcs):
                    bias_tile = []
                    simps = []
                    for h in range(HPC):
                        bt = biasp.tile([128, 512], BF16, name=f"bias_tile{h}",
                                        tag="bias_tile")
                        nc.sync.dma_start(
                            bt[:],
                            BIAST[h, jc * 128:(jc + 1) * 128,
                                  ic * 512:(ic + 1) * 512])
                        bias_tile.append(bt)
                        simps.append(psp.tile([128, N], F32, name=f"simps{h}",
                                              tag="simps", bufs=2))
                    # sims b-outer: consecutive MMs alternate row groups
                    for b in range(B):
                        for h in range(HPC):
                            nc.tensor.matmul(
                                simps[h][:, b * 512:(b + 1) * 512],
                                kT[b][h * 64:(h + 1) * 64,
                                      jc * 128:(jc + 1) * 128],
                                qT[b][h * 64:(h + 1) * 64,
                                      ic * 512:(ic + 1) * 512],
                                start=True, stop=False, skip_group_check=True)
                    for h in range(HPC):
                        for b in range(B):
                            nc.tensor.matmul(
                                simps[h][:, b * 512:(b + 1) * 512],
                                identb[:], bias_tile[h][:],
                                start=False, stop=True, skip_group_check=True)
                        expt = expp.tile([128, N], F16, name="expt",
                                         tag="expt")
                        nc.scalar.activation(
                            expt[:], simps[h][:],
                            mybir.ActivationFunctionType.Exp)
                        q = pend[h]
                        q.append((expt, jc, jj))
                        if len(q) > 1:
                            do_av(h, q.pop(0), False)
                for b in range(B):
                    pass
                for h in range(HPC):
                    q = pend[h]
                    while q:
                        do_av(h, q.pop(0), len(q) == 0)

                warm(12, tag="simps", width=512)
                for b in range(B):
                    for h in range(HPC):
                        # fast 1/sums, broadcast, multiply on DVE
                        sums_sb = smallp.tile([1, 512], F32, name="sums_sb")
                        nc.vector.tensor_copy(sums_sb[:], av[h][b][DH:DH + 1, :])
                        recip = smallp.tile([1, 512], F32, name="recip")
                        nc.vector.reciprocal_approx_fast(recip[:], sums_sb[:])
                        recipb = smallp.tile([DH, 512], F32, name="recipb")
                        nc.gpsimd.partition_broadcast(recipb[:], recip[:])
                        nstage = smallp.tile([DH, 512], F32R, name="nstage")
                        nc.vector.tensor_tensor(
                            nstage[:], av[h][b][0:DH, :], recipb[:],
                            mybir.AluOpType.mult)
                        nc.sync.dma_start(
                            outT[b][h * 64:(h + 1) * 64,
                                    ic * 512:(ic + 1) * 512],
                            nstage[:])
                    # project this batch's ready quarter right away
                    for ib in range(ic * 4, ic * 4 + 4):
                        out_proj_half(b, ib)

    nc.compile()
    _NC_CACHE = nc
    return nc


def _prep_inputs(x, mem_k, mem_v, pos_bias, Wq, Wkv, Wo):
    """Build per-core input maps (host-side sharding)."""
    x = np.ascontiguousarray(x, dtype=np.float32)
    xT = np.ascontiguousarray(x.transpose(0, 2, 1)).astype(np.float16)

    # masked, transposed, bf16 pos_bias for all heads: [16, JT, N]
    pb = np.ascontiguousarray(
        pos_bias[0].transpose(0, 2, 1)).astype(np.float32)     # [16, JT, N]
    jj = np.arange(JT)[:, None]
    ii = np.arange(N)[None, :]
    mask = jj > (ii + M)
    pb = np.where(mask[None], np.float32(NEG), pb)
    pb = pb.astype(ml_dtypes.bfloat16)

    ones = np.ones((B, NJ_MEM, 128, 1), dtype=np.float16)
    in_maps = []
    for c in range(NCORE):
        cs = 128 * c
        wq = np.ascontiguousarray(Wq[:, cs:cs + 128] * SCALE).astype(np.float16)
        wk = np.ascontiguousarray(Wkv[:, cs:cs + 128]).astype(np.float16)
        wv = np.ascontiguousarray(Wkv[:, DIM + cs:DIM + cs + 128]).astype(np.float16)
        wo = np.ascontiguousarray(Wo[cs:cs + 128, :], dtype=np.float32)
        mkT = np.ascontiguousarray(
            mem_k[:, :, cs:cs + 128].transpose(0, 2, 1)).astype(np.float16)
        mv_s = mem_v[:, :, cs:cs + 128].astype(np.float16).reshape(B, NJ_MEM, 128, 2, DH)
        mv = np.empty((B, HPC, NJ_MEM, 128, DH + 1), dtype=np.float16)
        for h in range(HPC):
            mv[:, h, :, :, 0:DH] = mv_s[:, :, :, h, :]
            mv[:, h, :, :, DH:] = ones
        biasT = np.ascontiguousarray(pb[2 * c:2 * c + 2])
        in_maps.append({
            "xT": xT,
            "ones_self": np.ones((128, HPC, NJ - NJ_MEM, 1), dtype=np.float16),
            "wq": wq, "wk": wk, "wv": wv, "wo": wo,
            "mkT": mkT,
            "mv": np.ascontiguousarray(mv),
            "biasT": biasT,
        })
    return in_maps


def kernel(x, mem_k, mem_v, pos_bias, Wq, Wkv, Wo, bo, **_kw):
    nc = _build()
    in_maps = _prep_inputs(
        np.asarray(x), np.asarray(mem_k), np.asarray(mem_v),
        np.asarray(pos_bias), np.asarray(Wq), np.asarray(Wkv), np.asarray(Wo))
    res = bass_utils.run_bass_kernel_spmd(nc, in_maps, core_ids=list(range(NCORE)))
    out = np.zeros((B, N, DIM), dtype=np.float64)
    for r in res.results:
        out += r["out"].astype(np.float64)
    out += np.asarray(bo, dtype=np.float64)[None, None, :]
    return out.astype(np.float32)


# revision 54
# speedup vs baseline: 1.0653x; 1.0653x over previous
"""Trainium2 Bass kernel for memory-augmented causal attention.

Reference computation (b=2, n=1024, m=1024 memory, 16 heads, d_head=64):
  q = (x @ Wq) * scale ; k,v = split(x @ Wkv) ; k = [mem_k; k] ; v = [mem_v; v]
  sim = q k^T + pos_bias ; causal mask on self part ; softmax ; out = attn v
  return out @ Wo + bo

Sharding: 16 heads across 8 cores (2 heads/core), both batches on every core
(pos_bias reused across batches on-chip).  Each core computes a partial
output (its heads' contribution through Wo rows); host sums the 8 partials.

All matmuls run as float32r (full-rate fp32 on TRN2 PE, ~1e-4 rms rounding).
pos_bias is pre-transposed/masked/bf16 on host and added to the logits in
PSUM via a bf16 identity matmul.  exp on ScalarE.  Softmax denominators come
from a ones-column appended to V (row 64 of the AV accumulation).
"""

import numpy as np
import ml_dtypes

import concourse.bass as bass
import concourse.mybir as mybir
import concourse.tile as tile
from concourse import bacc
from concourse import bass_utils
from concourse.masks import make_identity

F32 = mybir.dt.float32
F32R = mybir.dt.float32r
BF16 = mybir.dt.bfloat16
F16 = mybir.dt.float16

HEADS = 16
DH = 64               # head dim
B = 2                 # batch
N = 1024              # query length
M = 1024              # memory length
JT = N + M            # total key length
DIM = 1024
SCALE = DH ** -0.5
NCORE = 8
HPC = HEADS // NCORE  # heads per core = 2
NEG = -1.0e9          # mask value (exp -> 0 in fp32)

NKC = DIM // 128      # contraction chunks for projections = 8
NJ = JT // 128        # j chunks = 16
NJ_MEM = M // 128     # memory j chunks = 8
NIC = N // 512        # i chunks of 512 = 2


def _self_chunks(ic):
    # self j-chunk k (j0 = 1024 + 128k) unmasked for i-chunk ic iff
    # j0 <= 1023 + ic*512 + 1024  ->  128k <= ic*512 + 511
    return (ic * 512 + 511) // 128 + 1


def _unmasked_jcs(ic):
    return list(range(NJ_MEM)) + [NJ_MEM + k for k in range(min(8, _self_chunks(ic)))]


_NC_CACHE = None


def _build():
    global _NC_CACHE
    if _NC_CACHE is not None:
        return _NC_CACHE

    nc = bacc.Bacc("TRN2", target_bir_lowering=False, debug=False)

    XT = nc.dram_tensor("xT", [B, DIM, N], F16, kind="ExternalInput").ap()
    WQ = nc.dram_tensor("wq", [DIM, 128], F16, kind="ExternalInput").ap()
    WK = nc.dram_tensor("wk", [DIM, 128], F16, kind="ExternalInput").ap()
    WV = nc.dram_tensor("wv", [DIM, 128], F16, kind="ExternalInput").ap()
    WO = nc.dram_tensor("wo", [128, DIM], F32R, kind="ExternalInput").ap()
    MKT = nc.dram_tensor("mkT", [B, 128, M], F16, kind="ExternalInput").ap()
    MV = nc.dram_tensor("mv", [B, HPC, NJ_MEM, 128, DH + 1], F16,
                        kind="ExternalInput").ap()
    BIAST = nc.dram_tensor("biasT", [HPC, JT, N], BF16, kind="ExternalInput").ap()
    ONES = nc.dram_tensor("ones_self", [128, HPC, NJ - NJ_MEM, 1], F16,
                          kind="ExternalInput").ap()
    OUT = nc.dram_tensor("out", [B, N, DIM], F16, kind="ExternalOutput").ap()

    with tile.TileContext(nc) as tc:
        with tc.tile_pool(name="const", bufs=1) as cp, \
             tc.tile_pool(name="wts", bufs=1) as wp, \
             tc.tile_pool(name="xtp", bufs=16) as xtp, \
             tc.tile_pool(name="big", bufs=1) as bigp, \
             tc.tile_pool(name="stage", bufs=1) as stp, \
             tc.tile_pool(name="biasp", bufs=40) as biasp, \
             tc.tile_pool(name="expp", bufs=16) as expp, \
             tc.tile_pool(name="outst", bufs=4) as outstp, \
             tc.tile_pool(name="smalls", bufs=3) as smallp, \
             tc.tile_pool(name="psum", bufs=1, space="PSUM") as psp:

            # ---- constants ----
            identb = cp.tile([128, 128], BF16)
            make_identity(nc, identb[:])
            identf = cp.tile([128, 128], F32)
            make_identity(nc, identf[:])
            identr = cp.tile([128, 128], F32R)
            nc.vector.tensor_copy(identr[:], identf[:])
            identh = cp.tile([128, 128], F16)
            nc.vector.tensor_copy(identh[:], identf[:])

            # ---- weights (scalar queue; sync busy with xT) ----
            wq_t = wp.tile([128, NKC * 128], F16, tag="wqo")
            wk_t = wp.tile([128, NKC * 128], F16)
            wv_t = wp.tile([128, NKC * 128], F16)
            wo_t = wp.tile([128, DIM], F32R, tag="wqo")
            def load_w(tl, src):
                nc.scalar.dma_start(
                    tl[:].rearrange("p (kc m) -> p kc m", m=128),
                    src.rearrange("(kc p) m -> p kc m", p=128))
            load_w(wq_t, WQ)

            # ---- persistent per-batch tensors ----
            qT = [bigp.tile([128, N], F16, name=f"qT{b}") for b in range(B)]
            kT = [bigp.tile([128, JT], F16, name=f"kT{b}") for b in range(B)]
            vaug = [bigp.tile([128, HPC * NJ * (DH + 1)], F16, name=f"vaug{b}")
                    for b in range(B)]

            def vaug_slice(b, h, jc):
                o = (h * NJ + jc) * (DH + 1)
                return vaug[b][:, o:o + DH + 1]
            outT = [bigp.tile([128, N], F32R, name=f"outT{b}") for b in range(B)]

            # =============== Phase 1: projections ===============
            copy_idx = 0

            def copy_balanced(out_ap, in_ap, eng=None):
                nonlocal copy_idx
                if eng is None:
                    eng = "v" if copy_idx % 2 == 0 else "s"
                    copy_idx += 1
                if eng == "v":
                    nc.vector.tensor_copy(out_ap, in_ap)
                else:
                    nc.scalar.copy(out_ap, in_ap)

            def warm(n, tag="smallps", width=128):
                # dummy matmuls on resident constants; positioned before a
                # known PE stall they keep the HAM clock at 2.4 GHz
                wps = psp.tile([128, width], F32, name="warmps", tag=tag,
                               bufs=4 if tag == "smallps" else 2)
                for _ in range(n):
                    nc.tensor.matmul(wps[:, 0:128], identr[:], identr[:],
                                     start=True, stop=True,
                                     skip_group_check=True)

            # preload all xT tiles for both batches (sync + scalar queues);
            # each weight is queued on scalar right before the batch needing it
            xts = {}
            for b in range(B):
                for kc in range(NKC):
                    t = xtp.tile([128, N], F16, name=f"xt{b}_{kc}", tag="xt")
                    eng = nc.sync if (kc % 2 == 0) else nc.scalar
                    eng.dma_start(t[:], XT[b, kc * 128:(kc + 1) * 128, :])
                    xts[(b, kc)] = t
                if b == 0:
                    load_w(wk_t, WK)
            load_w(wv_t, WV)
            nc.scalar.dma_start(wo_t[:], WO)

            for b in range(B):
                # mem parts straight from DRAM
                nc.sync.dma_start(kT[b][:, 0:M], MKT[b])
                for h in range(HPC):
                    nc.gpsimd.dma_start(
                        vaug[b][:].rearrange(
                            "p (h jc x) -> p h jc x", h=HPC, x=DH + 1)[:, h, 0:NJ_MEM],
                        MV[b, h].rearrange("jc p x -> p jc x"))
                # ones columns for the self chunks, via strided DMA
                for h in range(HPC):
                    nc.gpsimd.dma_start(
                        vaug[b][:].rearrange(
                            "p (s x) -> p s x", x=DH + 1)[
                            :, h * NJ + NJ_MEM:h * NJ + NJ, DH:DH + 1],
                        ONES[:, h])

            def proj_qk(kind, b):
                wt = wq_t if kind == "q" else wk_t
                ps = psp.tile([128, N], F32, name="projps", tag="simps", bufs=2)
                for icx in range(NIC):
                    for kc in range(NKC):
                        nc.tensor.matmul(
                            ps[:, icx * 512:(icx + 1) * 512],
                            wt[:, kc * 128:(kc + 1) * 128],
                            xts[(b, kc)][:, icx * 512:(icx + 1) * 512],
                            start=(kc == 0), stop=(kc == NKC - 1))
                if kind == "q":
                    copy_balanced(qT[b][:], ps[:], eng="v")
                else:
                    copy_balanced(kT[b][:, M:JT], ps[:], eng="v")

            def proj_v(b):
                # accumulate in [128,512] halves on smallps so the sims'
                # psum slots stay free (runs interleaved with attention)
                vst = stp.tile([128, N], F16, name="vstage")
                for icx in range(NIC):
                    ps = psp.tile([128, 512], F32, name="vps", tag="smallps",
                                  bufs=4)
                    for kc in range(NKC):
                        nc.tensor.matmul(
                            ps[:],
                            wv_t[:, kc * 128:(kc + 1) * 128],
                            xts[(b, kc)][:, icx * 512:(icx + 1) * 512],
                            start=(kc == 0), stop=(kc == NKC - 1))
                    copy_balanced(vst[:, icx * 512:(icx + 1) * 512], ps[:],
                                  eng="v")
                for jb in range(8):
                    tp = psp.tile([128, 128], F16, name="tps",
                                  tag="smallps", bufs=4)
                    nc.tensor.transpose(
                        tp[:], vst[:, jb * 128:(jb + 1) * 128], identh[:])
                    jc = NJ_MEM + jb
                    dst = vaug[b][:].rearrange(
                        "p (h jjc x) -> p h jjc x", h=HPC, x=DH + 1)[
                        :, :, jc, 0:DH]
                    copy_balanced(
                        dst, tp[:].rearrange("p (h x) -> p h x", h=HPC),
                        eng="v")

            warm(32)
            for kind, b in (("q", 0), ("k", 0), ("q", 1), ("k", 1)):
                proj_qk(kind, b)
                warm(4)
            proj_v(0)
            proj_v(1)

            # =============== Phase 2 + 3 interleaved ===============
            def out_proj_half(b, ib):
                ob = outstp.tile([128, DIM], F16, name="ob")
                for dc in range(DIM // 512):
                    ps = psp.tile([128, 512], F32, name="ops", tag="smallps",
                                  bufs=4)
                    nc.tensor.matmul(
                        ps[:],
                        outT[b][:, ib * 128:(ib + 1) * 128],
                        wo_t[:, dc * 512:(dc + 1) * 512],
                        start=True, stop=True)
                    copy_balanced(ob[:, dc * 512:(dc + 1) * 512], ps[:])
                eng = nc.sync if ib % 2 == 0 else nc.scalar
                eng.dma_start(OUT[b, ib * 128:(ib + 1) * 128, :], ob[:])

            for ic in range(NIC):
                jcs = _unmasked_jcs(ic)
                av = []  # allocated lazily at the first AV emission
                pend = [[] for _ in range(HPC)]  # AV two iterations behind

                def do_av(h, p, last):
                    if not av:
                        for hh in range(HPC):
                            av.append([psp.tile(
                                [DH + 1, 512], F32, name=f"av{hh}_{bb}",
                                tag="smallps", bufs=4) for bb in range(B)])
                    expt_, jc_, idx = p
                    for b in range(B):
                        nc.tensor.matmul(
                            av[h][b][:],
                            vaug_slice(b, h, jc_),
                            expt_[:, b * 512:(b + 1) * 512],
                            start=(idx == 0), stop=last,
                            skip_group_check=True)

                for jj, jc in enumerate(jcs):
                    bias_tile = []
                    simps = []
                    for h in range(HPC):
                        bt = biasp.tile([128, 512], BF16, name=f"bias_tile{h}",
                                        tag="bias_tile")
                        nc.sync.dma_start(
                            bt[:],
                            BIAST[h, jc * 128:(jc + 1) * 128,
                                  ic * 512:(ic + 1) * 512])
                        bias_tile.append(bt)
                        simps.append(psp.tile([128, N], F32, name=f"simps{h}",
                                              tag="simps", bufs=2))
                    # sims b-outer: consecutive MMs alternate row groups
                    for b in range(B):
                        for h in range(HPC):
                            nc.tensor.matmul(
                                simps[h][:, b * 512:(b + 1) * 512],
                                kT[b][h * 64:(h + 1) * 64,
                                      jc * 128:(jc + 1) * 128],
                                qT[b][h * 64:(h + 1) * 64,
                                      ic * 512:(ic + 1) * 512],
                                start=True, stop=False, skip_group_check=True)
                    for h in range(HPC):
                        for b in range(B):
                            nc.tensor.matmul(
                                simps[h][:, b * 512:(b + 1) * 512],
                                identb[:], bias_tile[h][:],
                                start=False, stop=True, skip_group_check=True)
                        expt = expp.tile([128, N], F16, name="expt",
                                         tag="expt")
                        nc.scalar.activation(
                            expt[:], simps[h][:],
                            mybir.ActivationFunctionType.Exp)
                        q = pend[h]
                        q.append((expt, jc, jj))
                        if len(q) > 1:
                            do_av(h, q.pop(0), False)
                for b in range(B):
                    pass
                for h in range(HPC):
                    q = pend[h]
                    while q:
                        do_av(h, q.pop(0), len(q) == 0)

                warm(12, tag="simps", width=512)
                for b in range(B):
                    for h in range(HPC):
                        # fast 1/sums, broadcast, multiply on DVE
                        sums_sb = smallp.tile([1, 512], F32, name="sums_sb")
                        nc.scalar.copy(sums_sb[:], av[h][b][DH:DH + 1, :])
                        recip = smallp.tile([1, 512], F32, name="recip")
                        nc.vector.reciprocal_approx_fast(recip[:], sums_sb[:])
                        recipb = smallp.tile([DH, 512], F32, name="recipb")
                        nc.gpsimd.partition_broadcast(recipb[:], recip[:])
                        nstage = smallp.tile([DH, 512], F32R, name="nstage")
                        nc.vector.tensor_tensor(
                            nstage[:], av[h][b][0:DH, :], recipb[:],
                            mybir.AluOpType.mult)
                        nc.sync.dma_start(
                            outT[b][h * 64:(h + 1) * 64,
                                    ic * 512:(ic + 1) * 512],
                            nstage[:])
                    # project this batch's ready quarter right away
                    for ib in range(ic * 4, ic * 4 + 4):
                        out_proj_half(b, ib)

    nc.compile()
    _NC_CACHE = nc
    return nc


def _prep_inputs(x, mem_k, mem_v, pos_bias, Wq, Wkv, Wo):
    """Build per-core input maps (host-side sharding)."""
    x = np.ascontiguousarray(x, dtype=np.float32)
    xT = np.ascontiguousarray(x.transpose(0, 2, 1)).astype(np.float16)

    # masked, transposed, bf16 pos_bias for all heads: [16, JT, N]
    pb = np.ascontiguousarray(
        pos_bias[0].transpose(0, 2, 1)).astype(np.float32)     # [16, JT, N]
    jj = np.arange(JT)[:, None]
    ii = np.arange(N)[None, :]
    mask = jj > (ii + M)
    pb = np.where(mask[None], np.float32(NEG), pb)
    pb = pb.astype(ml_dtypes.bfloat16)

    ones = np.ones((B, NJ_MEM, 128, 1), dtype=np.float16)
    in_maps = []
    for c in range(NCORE):
        cs = 128 * c
        wq = np.ascontiguousarray(Wq[:, cs:cs + 128] * SCALE).astype(np.float16)
        wk = np.ascontiguousarray(Wkv[:, cs:cs + 128]).astype(np.float16)
        wv = np.ascontiguousarray(Wkv[:, DIM + cs:DIM + cs + 128]).astype(np.float16)
        wo = np.ascontiguousarray(Wo[cs:cs + 128, :], dtype=np.float32)
        mkT = np.ascontiguousarray(
            mem_k[:, :, cs:cs + 128].transpose(0, 2, 1)).astype(np.float16)
        mv_s = mem_v[:, :, cs:cs + 128].astype(np.float16).reshape(B, NJ_MEM, 128, 2, DH)
        mv = np.empty((B, HPC, NJ_MEM, 128, DH + 1), dtype=np.float16)
        for h in range(HPC):
            mv[:, h, :, :, 0:DH] = mv_s[:, :, :, h, :]
            mv[:, h, :, :, DH:] = ones
        biasT = np.ascontiguousarray(pb[2 * c:2 * c + 2])
        in_maps.append({
            "xT": xT,
            "ones_self": np.ones((128, HPC, NJ - NJ_MEM, 1), dtype=np.float16),
            "wq": wq, "wk": wk, "wv": wv, "wo": wo,
            "mkT": mkT,
            "mv": np.ascontiguousarray(mv),
            "biasT": biasT,
        })
    return in_maps


def kernel(x, mem_k, mem_v, pos_bias, Wq, Wkv, Wo, bo, **_kw):
    nc = _build()
    in_maps = _prep_inputs(
        np.asarray(x), np.asarray(mem_k), np.asarray(mem_v),
        np.asarray(pos_bias), np.asarray(Wq), np.asarray(Wkv), np.asarray(Wo))
    res = bass_utils.run_bass_kernel_spmd(nc, in_maps, core_ids=list(range(NCORE)))
    out = np.zeros((B, N, DIM), dtype=np.float64)
    for r in res.results:
        out += r["out"].astype(np.float64)
    out += np.asarray(bo, dtype=np.float64)[None, None, :]
    return out.astype(np.float32)
